# revision 45
# baseline (speedup 1.0000x reference)
"""DSFourierAttention Trainium2 kernel.

Math (per (b, h) slice, validated vs the jax reference in fp64/bf16 model):
    qf = rfft(q*0.125, ortho) etc. as dense DFT matmuls (Fre/Fim [L, 512]).
    x (query) bins 0..511 kept; bin 512 dropped from the OUTPUT side only
    (adds ~9e-4 rel err, removes all ragged x handling).
    y (key) bins 0..512 kept in full: y=512 handled as a rank-1 row
    (dropping it costs 4.7e-2 — attention is near-uniform so the y=512
    term is coherent across x).
    qk_re[y, x] = kstk . qstk   (stacked [kfr;kfi] x [qfr;qfi], K=128)
    qk_im'[y, x] = kstk . qswp  (qswp = [-qfi; qfr]; sign-free under ^2)
    p = exp(sqrt(re^2 + im^2)) in bf16 (no max subtraction; |qk| <= ~5)
    qkv[x, e] = (p^T @ [vfr | vfi | ones]) / colsum  (ones col = colsum)
    out[l, e] = Gre^T @ qkvr + Gim^T @ qkvi   (irfft rows 0..511, w=[1,2..2])
    out = out * tau[b] + delta[b, l]

Sharding: batch-parallel, 2 batches per core across 8 cores.
All inputs host-relayouted so every DMA moves >=2KB-contiguous partition
rows. ACT phases are batched per batch (sq -> sqrt -> exp) to amortize
activation-table loads; iFFT(b-1) and v-FFT(b) are interleaved to cover
the softmax ACT phases with PE work.
"""

import os
import sys

import numpy as np

for _p in ("/opt/trn_rl_repo", "/root/.axon_site/_ro/trn_rl_repo"):
    if os.path.isdir(_p) and _p not in sys.path:
        sys.path.insert(0, _p)

import ml_dtypes  # noqa: E402
import concourse.bass as bass  # noqa: E402
import concourse.tile as tile  # noqa: E402
from concourse import bacc, mybir  # noqa: E402
from concourse.bass_utils import run_bass_kernel_spmd  # noqa: E402

B, L, H, E = 16, 1024, 8, 64
X = 512                 # query-side bins kept (bin 512 dropped)
NCORES = 8
BL = B // NCORES        # 2 batches per core
NLC = L // 128          # 8 l-chunks
NYC = 4                 # dense 128-row y chunks (y=512 is a rank-1 row)
NXC = 4

F32 = mybir.dt.float32
BF16 = mybir.dt.bfloat16
AF = mybir.ActivationFunctionType

LAST_RESULT = None


PI = np.concatenate([np.arange(257), np.arange(511, 256, -1)])  # pos -> bin


def _consts():
    l = np.arange(L)
    xs = np.arange(X)
    ang = 2.0 * np.pi * np.outer(l, xs) / L              # [L, X]
    s = 1.0 / np.sqrt(L)
    fre = np.cos(ang) * s
    fim = -np.sin(ang) * s
    f512 = (((-1.0) ** l) * s).astype(ml_dtypes.bfloat16)  # [L]
    w = np.full(X, 2.0)
    w[0] = 1.0
    gre = w[:, None] * np.cos(ang.T) * s
    gim = w[:, None] * -np.sin(ang.T) * s
    # FFT consts: [128 p, 8 c, X] with row l = c*128+p
    fre16 = fre.astype(ml_dtypes.bfloat16)
    fim16 = fim.astype(ml_dtypes.bfloat16)
    fre_d = np.ascontiguousarray(fre16.reshape(NLC, 128, X).transpose(1, 0, 2))
    fim_d = np.ascontiguousarray(fim16.reshape(NLC, 128, X).transpose(1, 0, 2))
    f512_d = np.ascontiguousarray(f512.reshape(NLC, 128).transpose(1, 0)[:, 0:1])
    # iFFT consts: [128 p, 4 xc, L] with row x = xc*128+p
    gre16 = gre.astype(ml_dtypes.bfloat16)
    gim16 = gim.astype(ml_dtypes.bfloat16)
    gre_d = np.ascontiguousarray(gre16.reshape(NXC, 128, L).transpose(1, 0, 2))
    gim_d = np.ascontiguousarray(gim16.reshape(NXC, 128, L).transpose(1, 0, 2))
    return fre_d, fim_d, f512_d, gre_d, gim_d


def build_module(bl=BL, compile=True):
    from concourse.alu_op_type import AluOpType

    nc = bacc.Bacc("TRN2", target_bir_lowering=False, debug=False,
                   num_devices=NCORES)

    qd = nc.dram_tensor("qd", [bl, 4, 128, NLC, 128], BF16,
                        kind="ExternalInput").ap()
    kd = nc.dram_tensor("kd", [bl, 4, 128, NLC, 128], BF16,
                        kind="ExternalInput").ap()
    vd = nc.dram_tensor("vd", [bl, 128, NLC, 512], BF16,
                        kind="ExternalInput").ap()
    taud = nc.dram_tensor("taud", [bl, 1], F32, kind="ExternalInput").ap()
    deltad = nc.dram_tensor("deltad", [bl, 128, NLC], F32,
                            kind="ExternalInput").ap()
    fred = nc.dram_tensor("fred", [128, NLC, X], BF16, kind="ExternalInput").ap()
    fimd = nc.dram_tensor("fimd", [128, NLC, X], BF16, kind="ExternalInput").ap()
    f512d = nc.dram_tensor("f512d", [128, 1], BF16, kind="ExternalInput").ap()
    gred = nc.dram_tensor("gred", [128, NXC, L], BF16, kind="ExternalInput").ap()
    gimd = nc.dram_tensor("gimd", [128, NXC, L], BF16, kind="ExternalInput").ap()
    outd = nc.dram_tensor("outd", [bl, NLC, 128, 512], F32,
                          kind="ExternalOutput").ap()

    with tile.TileContext(nc) as tc:
        _body(nc, tc, AluOpType, qd, kd, vd, taud, deltad, fred, fimd, f512d,
              gred, gimd, outd, bl)
    if compile:
        nc.compile()
    return nc


def _body(nc, tc, OPS, qd, kd, vd, taud, deltad, fred, fimd, f512d,
          gred, gimd, outd, bl=BL):
    from contextlib import ExitStack

    ctx = ExitStack()
    with ctx:
        consts = ctx.enter_context(tc.tile_pool(name="consts", bufs=1))
        io = ctx.enter_context(tc.tile_pool(name="io", bufs=3))
        stg = ctx.enter_context(tc.tile_pool(name="stg", bufs=2))
        stk = ctx.enter_context(tc.tile_pool(name="stk", bufs=5))
        uvp = ctx.enter_context(tc.tile_pool(name="uvp", bufs=18))
        sqb = ctx.enter_context(tc.tile_pool(name="sqb", bufs=3))
        rg = ctx.enter_context(tc.tile_pool(name="rg", bufs=2))
        vfp = ctx.enter_context(tc.tile_pool(name="vfp", bufs=8))
        qkvp = ctx.enter_context(tc.tile_pool(name="qkvp", bufs=8))
        ep = ctx.enter_context(tc.tile_pool(name="ep", bufs=3))
        sm = ctx.enter_context(tc.tile_pool(name="sm", bufs=4))
        pf = ctx.enter_context(tc.tile_pool(name="pf", bufs=5, space="PSUM"))
        ph = ctx.enter_context(tc.tile_pool(name="ph", bufs=1, space="PSUM"))
        pm = ctx.enter_context(tc.tile_pool(name="pm", bufs=2, space="PSUM"))

        # ---- constants (fre/fim first: needed by the first FFT; split
        # across the two DMA-issuing queues so issue isn't serialized) ---
        fre_sb = consts.tile([128, NLC, X], BF16)
        fim_sb = consts.tile([128, NLC, X], BF16)
        f512_sb = consts.tile([128, 1], BF16)
        nc.gpsimd.dma_start(out=f512_sb[:, :], in_=f512d[:, :])
        for c4 in range(0, NLC, 4):
            nc.sync.dma_start(out=fre_sb[:, c4:c4 + 4, :],
                              in_=fred[:, c4:c4 + 4, :])
            nc.gpsimd.dma_start(out=fim_sb[:, c4:c4 + 4, :],
                                in_=fimd[:, c4:c4 + 4, :])
        gre_sb = consts.tile([128, NXC, L], BF16)
        gim_sb = consts.tile([128, NXC, L], BF16)

        # HAM warm-up: PE busy during the input-DMA window so the clock
        # gate is already at 8/8 when the first real matmul issues.
        wua = sm.tile([128, 128], BF16, tag="wua", bufs=1, name="wua")
        nc.vector.memset(wua[:, :], 0.0)
        wub = sm.tile([128, 128], BF16, tag="wub", bufs=1, name="wub")
        nc.vector.memset(wub[:, :], 0.0)
        ps_wu = pm.tile([128, 512], F32, tag="pm", name="pswu")
        for i in range(45):
            nc.tensor.matmul(ps_wu[:, 0:128], wua[:, :], wub[:, :],
                             start=(i == 0), stop=(i == 44))
        wu2 = sm.tile([128, 1], F32, tag="wu2", bufs=1, name="wu2")
        nc.vector.tensor_copy(out=wu2[:, :], in_=ps_wu[:, 0:1])

        state = {}
        for b in range(bl):
            # waves: FFT + QK + squares (q/k DMAs issue first inside)
            u_tiles = {}
            rag_u8 = rg.tile([8, X], BF16, tag="ragu", name=f"ragu{b}")
            v_sb = io.tile([128, NLC, 512], BF16, tag="vsb", bufs=2,
                           name=f"vsb{b}")
            tau_sb = ep.tile([128, 1], F32, tag="tau", bufs=2, name=f"tau{b}")
            delta_sb = ep.tile([128, NLC], F32, tag="delta", bufs=2,
                               name=f"delta{b}")
            v_ems = _phase_v_emitters(nc, b, v_sb, fre_sb, fim_sb,
                                      f512_sb, vfp, pf, pm)
            if b > 0:
                if_ems = _ifft_emitters(nc, OPS, b - 1, state[b - 1],
                                        gre_sb, gim_sb, outd, ep, pf)
                fillers = [x for pair in zip(v_ems[:8], if_ems)
                           for x in pair] + v_ems[8:]
            else:
                fillers = v_ems
            for w in range(2):
                _wave(nc, OPS, b, w, qd, kd, fre_sb, fim_sb, f512_sb,
                      io, stg, stk, uvp, sqb, rg, sm, pf, pm,
                      u_tiles, rag_u8, fillers)
                if w == 0:
                    # bulk DMAs issue behind the first wave's q/k loads
                    for c2 in range(0, NLC, 2):
                        nc.sync.dma_start(out=v_sb[:, c2:c2 + 2, :],
                                          in_=vd[b, :, c2:c2 + 2, :])
                    nc.sync.dma_start(
                        out=tau_sb[:, :],
                        in_=taud[b:b + 1, 0:1].to_broadcast([128, 1]))
                    nc.sync.dma_start(out=delta_sb[:, :], in_=deltad[b])
                    if b == 0:
                        for x2 in range(NXC):
                            nc.sync.dma_start(out=gre_sb[:, x2:x2 + 1, :],
                                              in_=gred[:, x2:x2 + 1, :])
                            nc.gpsimd.dma_start(out=gim_sb[:, x2:x2 + 1, :],
                                                in_=gimd[:, x2:x2 + 1, :])
            # flush unconsumed fillers (v-rag + remaining iFFT blocks)
            for f in fillers:
                f()
            del fillers[:]
            vf_av, v512 = _VRES[b]

            # ---- batched ACT phases: sqrt then exp (paired [128, 1024]
            # tiles across head-pairs to halve ACT op count) --------------
            for hp in range(4):
                for yc in range(NYC):
                    t = u_tiles[(hp, yc)]
                    nc.scalar.sqrt(out=t[:, :], in_=t[:, :])
            nc.scalar.sqrt(out=rag_u8[:, :], in_=rag_u8[:, :])
            for hp in range(4):
                for yc in range(NYC):
                    t = u_tiles[(hp, yc)]
                    nc.scalar.activation(out=t[:, :], in_=t[:, :], func=AF.Exp)
            nc.scalar.activation(out=rag_u8[:, :], in_=rag_u8[:, :],
                                 func=AF.Exp)
            exp_rag = {}
            for h in range(H):
                er = rg.tile([1, X], BF16, tag="exprag", bufs=16,
                             name=f"er{b}_{h}")
                exp_rag[h] = er
                nc.gpsimd.dma_start(out=er[0:1, :], in_=rag_u8[h:h + 1, :])

            # ---- AV ----------------------------------------------------
            qkv_all = [qkvp.tile([128, 2, H, 64], BF16, tag="qkv",
                                 name=f"qkv{b}_{xc}") for xc in range(NXC)]
            for h in range(H):
                for xc in range(NXC):
                    xcs = slice(xc * 128, (xc + 1) * 128)
                    # alternate between two pools for PSUM depth 3
                    avp = (ph, pm)[(h * NXC + xc) % 2]
                    ps_av = avp.tile([128, 512], F32,
                                     tag="ph" if avp is ph else "pm",
                                     name=f"psav{b}_{h}_{xc}")
                    for yc in range(NYC):
                        u2 = u_tiles[(h // 2, yc)]
                        off = X * (h % 2)
                        nc.tensor.matmul(
                            ps_av[:, 0:129],
                            u2[:, off + xc * 128:off + (xc + 1) * 128],
                            vf_av[yc][:, h, 0:129],
                            start=(yc == 0), stop=False)
                    nc.tensor.matmul(ps_av[:, 0:129], exp_rag[h][0:1, xcs],
                                     v512[0:1, h, 0:129],
                                     start=False, stop=True)
                    rc = sm.tile([128, 1], F32, tag="rc", bufs=4,
                                 name=f"rc{b}_{h}_{xc}")
                    nc.vector.reciprocal(out=rc[:, :], in_=ps_av[:, 128:129])
                    nc.vector.tensor_scalar_mul(
                        out=qkv_all[xc][:, :, h, :],
                        in0=ps_av[:, 0:128].rearrange("p (t e) -> p t e", t=2),
                        scalar1=rc[:, 0:1])
            state[b] = (qkv_all, tau_sb, delta_sb)

        _ifft(nc, OPS, bl - 1, state[bl - 1], gre_sb, gim_sb, outd, ep, pf)


def _wave(nc, OPS, b, w, qd, kd, fre_sb, fim_sb, f512_sb,
          io, stg, stk, uvp, sqb, rg, sm, pf, pm, u_tiles, rag_u8,
          fillers=None):
    hps = [2 * w, 2 * w + 1]
    qstk = {}
    qswp = {}
    kstk = {}
    k2col = {}

    for hp in hps:
        q_hp = io.tile([128, NLC, 128], BF16, tag="qhp", name=f"qhp{b}_{hp}")
        nc.sync.dma_start(out=q_hp[:, :, :], in_=qd[b, hp])
        k_hp = io.tile([128, NLC, 128], BF16, tag="khp", name=f"khp{b}_{hp}")
        nc.gpsimd.dma_start(out=k_hp[:, :, :], in_=kd[b, hp])

        # ---- q FFT -----------------------------------------------------
        ps_qr = pf.tile([128, 512], F32, tag="pf", name=f"psqr{b}_{hp}")
        ps_qi = pf.tile([128, 512], F32, tag="pf", name=f"psqi{b}_{hp}")
        for c in range(NLC):
            nc.tensor.matmul(ps_qr[:, 0:512], q_hp[:, c, :], fre_sb[:, c, :],
                             start=(c == 0), stop=(c == NLC - 1))
            nc.tensor.matmul(ps_qi[:, 0:512], q_hp[:, c, :], fim_sb[:, c, :],
                             start=(c == 0), stop=(c == NLC - 1))
        st_re = stg.tile([128, X], BF16, tag="stre", name=f"stre{b}_{hp}")
        nc.vector.tensor_copy(out=st_re[:, :], in_=ps_qr[:, 0:512])
        st_im = stg.tile([128, X], BF16, tag="stim", name=f"stim{b}_{hp}")
        nc.vector.tensor_copy(out=st_im[:, :], in_=ps_qi[:, 0:512])
        st_imn = stg.tile([128, X], BF16, tag="stimn", name=f"stimn{b}_{hp}")
        nc.vector.tensor_scalar_mul(out=st_imn[:, :], in0=ps_qi[:, 0:512],
                                    scalar1=-1.0)

        # ---- k FFT + bin-512 column ------------------------------------
        ps_kr = pf.tile([128, 512], F32, tag="pf", name=f"pskr{b}_{hp}")
        ps_ki = pf.tile([128, 512], F32, tag="pf", name=f"pski{b}_{hp}")
        ps_kc = pm.tile([128, 512], F32, tag="pm", name=f"pskc{b}_{hp}")
        for c in range(NLC):
            nc.tensor.matmul(ps_kr[:, 0:512], k_hp[:, c, :], fre_sb[:, c, :],
                             start=(c == 0), stop=(c == NLC - 1))
            nc.tensor.matmul(ps_kc[:, 0:1], k_hp[:, c, :], f512_sb[:, 0:1],
                             start=(c == 0), stop=(c == NLC - 1))
            nc.tensor.matmul(ps_ki[:, 0:512], k_hp[:, c, :], fim_sb[:, c, :],
                             start=(c == 0), stop=(c == NLC - 1))
        kt_re = stg.tile([128, X], BF16, tag="ktre", name=f"ktre{b}_{hp}")
        nc.vector.tensor_copy(out=kt_re[:, :], in_=ps_kr[:, 0:512])
        kt_im = stg.tile([128, X], BF16, tag="ktim", name=f"ktim{b}_{hp}")
        nc.vector.tensor_copy(out=kt_im[:, :], in_=ps_ki[:, 0:512])
        kc_sb = sm.tile([128, 1], BF16, tag="kc", bufs=4, name=f"kc{b}_{hp}")
        nc.vector.tensor_copy(out=kc_sb[:, :], in_=ps_kc[:, 0:1])

        # ---- stacks (SBUF->SBUF partition moves, split across the SP
        # and Pool DMA queues to halve per-sequencer trigger load) -------
        for phi in range(2):
            h = 2 * hp + phi
            rows = slice(64 * phi, 64 * phi + 64)
            dq = (nc.gpsimd, nc.sync)[phi]
            qs = stk.tile([128, X], BF16, tag="qstk", name=f"qstk{b}_{h}")
            qstk[h] = qs
            dq.dma_start(out=qs[0:64, :], in_=st_re[rows, :])
            dq.dma_start(out=qs[64:128, :], in_=st_im[rows, :])
            qw = stk.tile([128, X], BF16, tag="qswp", name=f"qswp{b}_{h}")
            qswp[h] = qw
            dq.dma_start(out=qw[0:64, :], in_=st_imn[rows, :])
            dq.dma_start(out=qw[64:128, :], in_=st_re[rows, :])
            ks = stk.tile([128, X], BF16, tag="kstk", name=f"kstk{b}_{h}")
            kstk[h] = ks
            dq.dma_start(out=ks[0:64, :], in_=kt_re[rows, :])
            dq.dma_start(out=ks[64:128, :], in_=kt_im[rows, :])
            kc = stk.tile([128, 2], BF16, tag="k2col", bufs=8,
                          name=f"k2col{b}_{h}")
            k2col[h] = kc
            nc.vector.memset(kc[:, :], 0.0)
            dq.dma_start(out=kc[0:64, 0:1], in_=kc_sb[rows, :])
            dq.dma_start(out=kc[64:128, 1:2], in_=kc_sb[rows, :])

    # ---- QK + squares (square is in every ACT table set) ---------------
    for hp in hps:
        for phi in range(2):
            h = 2 * hp + phi
            for yc in range(NYC):
                ycs = slice(yc * 128, (yc + 1) * 128)
                ps_r = pf.tile([128, 512], F32, tag="pf",
                               name=f"psr{b}_{h}_{yc}")
                ps_i = pf.tile([128, 512], F32, tag="pf",
                               name=f"psi{b}_{h}_{yc}")
                nc.tensor.matmul(ps_r[:, 0:512], kstk[h][:, ycs],
                                 qstk[h][:, 0:512], start=True, stop=True)
                nc.tensor.matmul(ps_i[:, 0:512], kstk[h][:, ycs],
                                 qswp[h][:, 0:512], start=True, stop=True)
                if phi == 0:
                    u2 = uvp.tile([128, 2 * X], BF16, tag="u",
                                  name=f"u{b}_{hp}_{yc}")
                    u_tiles[(hp, yc)] = u2
                else:
                    u2 = u_tiles[(hp, yc)]
                wa = sqb.tile([128, X], BF16, tag="sqa",
                              name=f"sqa{b}_{h}_{yc}")
                wb = sqb.tile([128, X], BF16, tag="sqb",
                              name=f"sqb{b}_{h}_{yc}")
                nc.scalar.square(out=wa[:, :], in_=ps_r[:, 0:512])
                if yc % 2:
                    # ACT is the pacing engine mid-kernel: route half the
                    # im-squares through DVE (drain to bf16, then square)
                    cb = sqb.tile([128, X], BF16, tag="sqc",
                                  name=f"sqc{b}_{h}_{yc}")
                    nc.vector.tensor_copy(out=cb[:, :], in_=ps_i[:, 0:512])
                    nc.vector.tensor_mul(out=wb[:, :], in0=cb[:, :],
                                         in1=cb[:, :])
                else:
                    nc.scalar.square(out=wb[:, :], in_=ps_i[:, 0:512])
                nc.vector.tensor_add(out=u2[:, X * phi:X * phi + X],
                                     in0=wa[:, :], in1=wb[:, :])

            # rag y=512 scores: one M=2 matmul (re on row 0, im on row 1)
            ps_g = pm.tile([128, 512], F32, tag="pm", name=f"psg{b}_{h}")
            nc.tensor.matmul(ps_g[0:2, 0:512], k2col[h][:, 0:2],
                             qstk[h][:, 0:512], start=True, stop=True)
            r2 = rg.tile([2, X], BF16, tag="rag2", bufs=4, name=f"r2{b}_{h}")
            nc.scalar.square(out=r2[:, :], in_=ps_g[0:2, 0:512])
            rb = rg.tile([1, X], BF16, tag="ragb", bufs=4, name=f"rb{b}_{h}")
            nc.gpsimd.dma_start(out=rb[0:1, :], in_=r2[1:2, :])
            ru = rg.tile([1, X], BF16, tag="ragu1", bufs=4, name=f"ru{b}_{h}")
            nc.vector.tensor_add(out=ru[0:1, :], in0=r2[0:1, :], in1=rb[0:1, :])
            nc.gpsimd.dma_start(out=rag_u8[h:h + 1, :], in_=ru[0:1, :])
            if fillers:
                fillers.pop(0)()


_VRES = {}


def _phase_v_emitters(nc, b, v_sb, fre_sb, fim_sb, f512_sb, vfp, pf, pm):
    vf_av = [vfp.tile([128, H, 132], BF16, tag="vfav", bufs=5,
                      name=f"vfav{b}_{yc}") for yc in range(NYC)]
    v512 = vfp.tile([1, H, 132], BF16, tag="v512", bufs=2, name=f"v512_{b}")
    _VRES[b] = (vf_av, v512)
    ems = []
    for part, f_sb in ((0, fre_sb), (1, fim_sb)):
        for yc in range(NYC):
            def em(part=part, f_sb=f_sb, yc=yc):
                ps = pf.tile([128, 512], F32, tag="pf",
                             name=f"psv{b}_{part}_{yc}")
                for c in range(NLC):
                    nc.tensor.matmul(ps[:, 0:512],
                                     f_sb[:, c, yc * 128:(yc + 1) * 128],
                                     v_sb[:, c, :],
                                     start=(c == 0), stop=(c == NLC - 1))
                nc.vector.tensor_copy(
                    out=vf_av[yc][:, :, part * 64:(part + 1) * 64],
                    in_=ps[:, 0:512].rearrange("p (h e) -> p h e", h=H))
                if part == 1:
                    nc.vector.memset(vf_av[yc][:, :, 128:129], 1.0)
            ems.append(em)

    def em_rag():
        ps512 = pm.tile([128, 512], F32, tag="pm", name=f"psv512_{b}")
        for c in range(NLC):
            nc.tensor.matmul(ps512[0:1, 0:512], f512_sb[:, 0:1],
                             v_sb[:, c, :],
                             start=(c == 0), stop=(c == NLC - 1))
        nc.vector.tensor_copy(
            out=v512[0:1, :, 0:64],
            in_=ps512[0:1, 0:512].rearrange("p (h e) -> p h e", h=H))
        nc.vector.memset(v512[0:1, :, 64:128], 0.0)
        nc.vector.memset(v512[0:1, :, 128:129], 1.0)
    ems.append(em_rag)
    return ems


def _ifft_emitters(nc, OPS, b, st, gre_sb, gim_sb, outd, ep, pf):
    qkv_all, tau_sb, delta_sb = st
    ems = []
    for lc in range(NLC):
        def em(lc=lc):
            lcs = slice(lc * 128, (lc + 1) * 128)
            ps_o = pf.tile([128, 512], F32, tag="pf", name=f"pso{b}_{lc}")
            for xc in range(NXC):
                nc.tensor.matmul(ps_o[:, 0:512], gre_sb[:, xc, lcs],
                                 qkv_all[xc][:, 0, :, :],
                                 start=(xc == 0), stop=False)
                nc.tensor.matmul(ps_o[:, 0:512], gim_sb[:, xc, lcs],
                                 qkv_all[xc][:, 1, :, :],
                                 start=False, stop=(xc == NXC - 1))
            out_t = ep.tile([128, 512], F32, tag="outsb", name=f"out{b}_{lc}")
            nc.vector.tensor_scalar(out=out_t[:, :], in0=ps_o[:, 0:512],
                                    scalar1=tau_sb[:, 0:1],
                                    scalar2=delta_sb[:, lc:lc + 1],
                                    op0=OPS.mult, op1=OPS.add)
            nc.sync.dma_start(out=outd[b, lc], in_=out_t[:, :])
        ems.append(em)
    return ems


def _phase_v(nc, b, v_sb, fre_sb, fim_sb, f512_sb, vfp, pf, pm):
    vf_av = []
    for yc in range(NYC):
        t = vfp.tile([128, H, 132], BF16, tag="vfav", bufs=5,
                     name=f"vfav{b}_{yc}")
        vf_av.append(t)
    v512 = vfp.tile([1, H, 132], BF16, tag="v512", bufs=2, name=f"v512_{b}")

    for part, f_sb in ((0, fre_sb), (1, fim_sb)):
        for yc in range(NYC):
            ps = pf.tile([128, 512], F32, tag="pf", name=f"psv{b}_{part}_{yc}")
            for c in range(NLC):
                nc.tensor.matmul(ps[:, 0:512],
                                 f_sb[:, c, yc * 128:(yc + 1) * 128],
                                 v_sb[:, c, :],
                                 start=(c == 0), stop=(c == NLC - 1))
            nc.vector.tensor_copy(
                out=vf_av[yc][:, :, part * 64:(part + 1) * 64],
                in_=ps[:, 0:512].rearrange("p (h e) -> p h e", h=H))
    for yc in range(NYC):
        nc.vector.memset(vf_av[yc][:, :, 128:129], 1.0)

    ps512 = pm.tile([128, 512], F32, tag="pm", name=f"psv512_{b}")
    for c in range(NLC):
        nc.tensor.matmul(ps512[0:1, 0:512], f512_sb[:, 0:1], v_sb[:, c, :],
                         start=(c == 0), stop=(c == NLC - 1))
    nc.vector.tensor_copy(out=v512[0:1, :, 0:64],
                          in_=ps512[0:1, 0:512].rearrange("p (h e) -> p h e",
                                                          h=H))
    nc.vector.memset(v512[0:1, :, 64:128], 0.0)
    nc.vector.memset(v512[0:1, :, 128:129], 1.0)
    return vf_av, v512


def _ifft(nc, OPS, b, st, gre_sb, gim_sb, outd, ep, pf):
    qkv_all, tau_sb, delta_sb = st
    for lc in range(NLC):
        lcs = slice(lc * 128, (lc + 1) * 128)
        ps_o = pf.tile([128, 512], F32, tag="pf", name=f"pso{b}_{lc}")
        for xc in range(NXC):
            nc.tensor.matmul(ps_o[:, 0:512], gre_sb[:, xc, lcs],
                             qkv_all[xc][:, 0, :, :],
                             start=(xc == 0), stop=False)
            nc.tensor.matmul(ps_o[:, 0:512], gim_sb[:, xc, lcs],
                             qkv_all[xc][:, 1, :, :],
                             start=False, stop=(xc == NXC - 1))
        out_t = ep.tile([128, 512], F32, tag="outsb", name=f"out{b}_{lc}")
        nc.vector.tensor_scalar(out=out_t[:, :], in0=ps_o[:, 0:512],
                                scalar1=tau_sb[:, 0:1],
                                scalar2=delta_sb[:, lc:lc + 1],
                                op0=OPS.mult, op1=OPS.add)
        nc.sync.dma_start(out=outd[b, lc], in_=out_t[:, :])


_BUILT = None
_CONSTS = None


def _get_built():
    global _BUILT, _CONSTS
    if _BUILT is None:
        _BUILT = build_module()
        _CONSTS = _consts()
    return _BUILT, _CONSTS


def kernel(q, k, v, mask, tau, delta):
    global LAST_RESULT
    nc, (fre_d, fim_d, f512_d, gre_d, gim_d) = _get_built()
    q = np.asarray(q, dtype=np.float32) * 0.125
    q = q.astype(ml_dtypes.bfloat16)
    k = np.asarray(k, dtype=np.float32).astype(ml_dtypes.bfloat16)
    v = np.asarray(v, dtype=np.float32).astype(ml_dtypes.bfloat16)
    tau = np.ascontiguousarray(np.asarray(tau, dtype=np.float32))
    delta = np.asarray(delta, dtype=np.float32)

    # host relayouts (all contiguous, >=2KB partition rows on device)
    def qk_layout(x):
        # [b, (c p), (hp phi), e] -> [b, hp, p, c, (phi e)]
        x = x.reshape(-1, NLC, 128, 4, 2, E)
        return np.ascontiguousarray(x.transpose(0, 3, 2, 1, 4, 5)
                                    .reshape(-1, 4, 128, NLC, 128))

    qh = qk_layout(q)
    kh = qk_layout(k)
    vh = np.ascontiguousarray(
        v.reshape(-1, NLC, 128, H * E).transpose(0, 2, 1, 3))
    dh = np.ascontiguousarray(
        delta.reshape(-1, NLC, 128).transpose(0, 2, 1))

    in_maps = []
    for i in range(NCORES):
        sl = slice(i * BL, (i + 1) * BL)
        in_maps.append({
            "qd": np.ascontiguousarray(qh[sl]),
            "kd": np.ascontiguousarray(kh[sl]),
            "vd": np.ascontiguousarray(vh[sl]),
            "taud": np.ascontiguousarray(tau[sl]),
            "deltad": np.ascontiguousarray(dh[sl]),
            "fred": fre_d, "fimd": fim_d, "f512d": f512_d,
            "gred": gre_d, "gimd": gim_d,
        })
    res = run_bass_kernel_spmd(nc, in_maps, core_ids=list(range(NCORES)))
    LAST_RESULT = res
    out = np.concatenate([res.results[i]["outd"] for i in range(NCORES)],
                         axis=0)
    return out.reshape(B, L, H, E).astype(np.float32)
